# revision 1
# baseline (speedup 1.0000x reference)
"""InterWindowAttn kernel for 8 trn2 NeuronCores.

Strategy (per sharding hint): shard the window axis N=B*gh*gw=4096 across the
8 cores (512 windows each) for QKV attention + LePE. The cluster-similarity /
top-k stage needs the whole batch, so the pooled descriptors (dsx) and the
window table (xw) are replicated to every core; each core computes its 512
rows of the similarity matrix against all 4096 windows, does the two-stage
top-k, gathers its top-3 neighbor windows locally, and runs the windowed
attention. Host only does layout transforms (window partition / reassembly).
"""

import time

import numpy as np
import jax
import jax.numpy as jnp

GS = 8
CS = 64
TOPK = 3
B, C, H, W = 4, 128, 256, 256
GH, GW = H // GS, W // GS          # 32, 32
N = B * GH * GW                    # 4096 windows
P = GS * GS                        # 64 pixels / window
NDEV = 8
NL = N // NDEV                     # 512 windows / core
ITR = N // CS                      # 64 clusters

LAST_EXEC_NS = None


def _fwd(xw_l, dsx_l, xw_f, dsx_f, wq, bq, wk, bk, wv, bv, wp, bp,
         lepe_w, lepe_b):
    # xw_l [NL,P,C] local windows; xw_f [N,P,C], dsx_f [N,C] replicated.
    # ---- similarity rows vs whole batch + two-stage top-k (ref order) ----
    sim = jnp.einsum('wc,vc->wv', dsx_l, dsx_f)          # [NL, N]
    simr = sim.reshape(NL, ITR, CS)                      # per-cluster chunks
    sc, idx = jax.lax.top_k(simr, TOPK)                  # [NL, ITR, K]
    gidx = idx + (jnp.arange(ITR) * CS)[None, :, None]   # global window ids
    fval, fpos = jax.lax.top_k(sc.reshape(NL, ITR * TOPK), TOPK)
    fidx = jnp.take_along_axis(gidx.reshape(NL, ITR * TOPK), fpos, axis=-1)

    # ---- weighted fusion of top-k neighbor windows ----
    w = jax.nn.softmax(fval, axis=-1)                    # [NL, K]
    neigh = xw_f[fidx]                                   # [NL, K, P, C]
    ctx = jnp.einsum('nk,nkpc->npc', w, neigh)           # [NL, P, C]

    # ---- windowed QKV attention with LePE ----
    q = xw_l @ wq.T + bq
    k = ctx @ wk.T + bk
    v = ctx @ wv.T + bv

    ximg = xw_l.reshape(NL, GS, GS, C).transpose(0, 3, 1, 2)
    lepe = jax.lax.conv_general_dilated(
        ximg, lepe_w, window_strides=(1, 1), padding='SAME',
        feature_group_count=C) + lepe_b[None, :, None, None]
    lepe = lepe.transpose(0, 2, 3, 1).reshape(NL, P, C)

    scale = C ** -0.5
    attn = jax.nn.softmax(jnp.einsum('npc,nqc->npq', q, k) * scale, axis=-1)
    out = jnp.einsum('npq,nqc->npc', attn, v) + lepe
    out = out @ wp.T + bp
    return out


_pfwd = jax.pmap(
    _fwd, axis_name='d',
    in_axes=(0, 0) + (None,) * 12,
)


def kernel(x, wq, bq, wk, bk, wv, bv, wp, bp, lepe_w, lepe_b):
    global LAST_EXEC_NS
    x = np.asarray(x, dtype=np.float32)

    # window partition on host: [B,C,H,W] -> [N,P,C]
    xw = np.ascontiguousarray(
        x.transpose(0, 2, 3, 1)
         .reshape(B, GH, GS, GW, GS, C)
         .transpose(0, 1, 3, 2, 4, 5)
         .reshape(N, P, C))
    # adaptive max pool on host (cheap): [N, C]
    dsx = xw.max(axis=1)

    xw_sh = xw.reshape(NDEV, NL, P, C)
    dsx_sh = dsx.reshape(NDEV, NL, C)

    t0 = time.perf_counter()
    out_sh = _pfwd(xw_sh, dsx_sh, xw, dsx,
                   wq, bq, wk, bk, wv, bv, wp, bp, lepe_w, lepe_b)
    out_sh = jax.block_until_ready(out_sh)
    LAST_EXEC_NS = (time.perf_counter() - t0) * 1e9

    out = np.asarray(out_sh).reshape(N, P, C)
    # reassemble: (b gh gw)(gs gs) c -> b c (gh gs)(gw gs)
    out = (out.reshape(B, GH, GW, GS, GS, C)
              .transpose(0, 5, 1, 3, 2, 4)
              .reshape(B, C, H, W))
    return np.ascontiguousarray(out.astype(np.float32))


# revision 2
# speedup vs baseline: 12.7759x; 12.7759x over previous
"""InterWindowAttn kernel for 8 trn2 NeuronCores.

Strategy (per sharding hint): shard the window axis N=B*gh*gw=4096 across the
8 cores (512 windows each, = one batch image half per core) for QKV attention
and LePE. The cluster-similarity / top-k stage needs the whole batch, so the
pooled descriptors (dsx) and the window table (xw) are all-gathered on-chip;
each core computes its 512 rows of the similarity matrix against all 4096
windows, does the two-stage top-k, gathers its top-3 neighbor windows from the
gathered table, and runs the windowed attention. Host only does cheap layout
transforms (slab split / reassembly).
"""

import time

import numpy as np
import jax
import jax.numpy as jnp

GS = 8
CS = 64
TOPK = 3
B, C, H, W = 4, 128, 256, 256
GH, GW = H // GS, W // GS          # 32, 32
N = B * GH * GW                    # 4096 windows
P = GS * GS                        # 64 pixels / window
NDEV = 8
NL = N // NDEV                     # 512 windows / core
GHL = GH // 2                      # 16 window-rows / core
ITR = N // CS                      # 64 clusters

LAST_EXEC_NS = None


def _fwd(xs, wq, bq, wk, bk, wv, bv, wp, bp, lepe_w, lepe_b):
    # xs [C, 128, W]: one batch-image half -> 512 local windows.
    # window partition on device: -> [NL, P, C]
    xw_l = (xs.transpose(1, 2, 0)
              .reshape(GHL, GS, GW, GS, C)
              .transpose(0, 2, 1, 3, 4)
              .reshape(NL, P, C))
    dsx_l = xw_l.max(axis=1)                             # [NL, C]

    # on-chip all-gathers (window order == device-major order)
    xw_f = jax.lax.all_gather(xs, 'd')                   # [8, C, 128, W]
    xw_f = (xw_f.transpose(0, 2, 3, 1)
                .reshape(NDEV * GHL, GS, GW, GS, C)
                .transpose(0, 2, 1, 3, 4)
                .reshape(N, P, C))
    dsx_f = jax.lax.all_gather(dsx_l, 'd').reshape(N, C)

    # ---- similarity rows vs whole batch + two-stage top-k (ref order) ----
    sim = jnp.einsum('wc,vc->wv', dsx_l, dsx_f)          # [NL, N]
    simr = sim.reshape(NL, ITR, CS)                      # per-cluster chunks
    sc, idx = jax.lax.top_k(simr, TOPK)                  # [NL, ITR, K]
    gidx = idx + (jnp.arange(ITR) * CS)[None, :, None]   # global window ids
    fval, fpos = jax.lax.top_k(sc.reshape(NL, ITR * TOPK), TOPK)
    fidx = jnp.take_along_axis(gidx.reshape(NL, ITR * TOPK), fpos, axis=-1)

    # ---- weighted fusion of top-k neighbor windows ----
    w = jax.nn.softmax(fval, axis=-1)                    # [NL, K]
    neigh = xw_f[fidx]                                   # [NL, K, P, C]
    ctx = jnp.einsum('nk,nkpc->npc', w, neigh)           # [NL, P, C]

    # ---- windowed QKV attention with LePE ----
    q = xw_l @ wq.T + bq
    k = ctx @ wk.T + bk
    v = ctx @ wv.T + bv

    ximg = xw_l.reshape(NL, GS, GS, C).transpose(0, 3, 1, 2)
    lepe = jax.lax.conv_general_dilated(
        ximg, lepe_w, window_strides=(1, 1), padding='SAME',
        feature_group_count=C) + lepe_b[None, :, None, None]
    lepe = lepe.transpose(0, 2, 3, 1).reshape(NL, P, C)

    scale = C ** -0.5
    attn = jax.nn.softmax(jnp.einsum('npc,nqc->npq', q, k) * scale, axis=-1)
    out = jnp.einsum('npq,nqc->npc', attn, v) + lepe
    out = out @ wp.T + bp
    return out


_pfwd = jax.pmap(
    _fwd, axis_name='d',
    in_axes=(0,) + (None,) * 10,
)


def kernel(x, wq, bq, wk, bk, wv, bv, wp, bp, lepe_w, lepe_b):
    global LAST_EXEC_NS
    x = np.asarray(x, dtype=np.float32)

    # shard raw x into 8 slabs: device d = (batch d//2, image half d%2)
    xs = np.ascontiguousarray(
        x.reshape(B, C, 2, H // 2, W).transpose(0, 2, 1, 3, 4)
         .reshape(NDEV, C, H // 2, W))

    t0 = time.perf_counter()
    out_sh = _pfwd(xs, wq, bq, wk, bk, wv, bv, wp, bp, lepe_w, lepe_b)
    out_sh = jax.block_until_ready(out_sh)
    LAST_EXEC_NS = (time.perf_counter() - t0) * 1e9

    out = np.asarray(out_sh).reshape(N, P, C)
    # reassemble: (b gh gw)(gs gs) c -> b c (gh gs)(gw gs)
    out = (out.reshape(B, GH, GW, GS, GS, C)
              .transpose(0, 5, 1, 3, 2, 4)
              .reshape(B, C, H, W))
    return np.ascontiguousarray(out.astype(np.float32))


# revision 4
# speedup vs baseline: 13.7830x; 1.0788x over previous
"""InterWindowAttn kernel for 8 trn2 NeuronCores.

Strategy (per sharding hint): shard the window axis N=B*gh*gw=4096 across the
8 cores (512 windows each, = one batch image half per core) for QKV attention
and LePE. The cluster-similarity / top-k stage needs the whole batch, so the
pooled descriptors (dsx) and the window table (xw) are all-gathered on-chip;
each core computes its 512 rows of the similarity matrix against all 4096
windows, does the two-stage top-k, gathers its top-3 neighbor windows from the
gathered table, and runs the windowed attention. Host only does cheap layout
transforms (slab split / reassembly).
"""

import time

import numpy as np
import jax
import jax.numpy as jnp

GS = 8
CS = 64
TOPK = 3
B, C, H, W = 4, 128, 256, 256
GH, GW = H // GS, W // GS          # 32, 32
N = B * GH * GW                    # 4096 windows
P = GS * GS                        # 64 pixels / window
NDEV = 8
NL = N // NDEV                     # 512 windows / core
GHL = GH // 2                      # 16 window-rows / core
ITR = N // CS                      # 64 clusters

LAST_EXEC_NS = None


def _fwd(xs, wq, bq, wk, bk, wv, bv, wp, bp, lepe_w, lepe_b):
    # xs [C, 128, W]: one batch-image half -> 512 local windows.
    # window partition on device: -> [NL, P, C]
    xw_l = (xs.transpose(1, 2, 0)
              .reshape(GHL, GS, GW, GS, C)
              .transpose(0, 2, 1, 3, 4)
              .reshape(NL, P, C))
    dsx_l = xw_l.max(axis=1)                             # [NL, C]

    # on-chip all-gathers (window order == device-major order)
    xw_f = jax.lax.all_gather(xs, 'd')                   # [8, C, 128, W]
    xw_f = (xw_f.transpose(0, 2, 3, 1)
                .reshape(NDEV * GHL, GS, GW, GS, C)
                .transpose(0, 2, 1, 3, 4)
                .reshape(N, P, C))
    dsx_f = jax.lax.all_gather(dsx_l, 'd').reshape(N, C)

    # ---- similarity rows vs whole batch + two-stage top-k (ref order) ----
    sim = jnp.einsum('wc,vc->wv', dsx_l, dsx_f)          # [NL, N]
    simr = sim.reshape(NL, ITR, CS)                      # per-cluster chunks
    sc, idx = jax.lax.top_k(simr, TOPK)                  # [NL, ITR, K]
    gidx = idx + (jnp.arange(ITR) * CS)[None, :, None]   # global window ids
    fval, fpos = jax.lax.top_k(sc.reshape(NL, ITR * TOPK), TOPK)
    fidx = jnp.take_along_axis(gidx.reshape(NL, ITR * TOPK), fpos, axis=-1)

    # ---- weighted fusion of top-k neighbor windows ----
    w = jax.nn.softmax(fval, axis=-1)                    # [NL, K]
    neigh = xw_f[fidx]                                   # [NL, K, P, C]
    ctx = jnp.einsum('nk,nkpc->npc', w, neigh)           # [NL, P, C]

    # ---- windowed QKV attention with LePE ----
    q = xw_l @ wq.T + bq
    k = ctx @ wk.T + bk
    v = ctx @ wv.T + bv

    ximg = xw_l.reshape(NL, GS, GS, C).transpose(0, 3, 1, 2)
    lepe = jax.lax.conv_general_dilated(
        ximg, lepe_w, window_strides=(1, 1), padding='SAME',
        feature_group_count=C) + lepe_b[None, :, None, None]
    lepe = lepe.transpose(0, 2, 3, 1).reshape(NL, P, C)

    scale = C ** -0.5
    attn = jax.nn.softmax(jnp.einsum('npc,nqc->npq', q, k) * scale, axis=-1)
    out = jnp.einsum('npq,nqc->npc', attn, v) + lepe
    out = out @ wp.T + bp
    # reassemble the local slab on device: [NL,P,C] -> [C, H/2, W]
    out = (out.reshape(GHL, GW, GS, GS, C)
              .transpose(4, 0, 2, 1, 3)
              .reshape(C, H // 2, W))
    return out


_pfwd = jax.pmap(
    _fwd, axis_name='d',
    in_axes=(0,) + (None,) * 10,
)


def kernel(x, wq, bq, wk, bk, wv, bv, wp, bp, lepe_w, lepe_b):
    global LAST_EXEC_NS
    x = np.asarray(x, dtype=np.float32)

    # shard raw x into 8 slabs: device d = (batch d//2, image half d%2)
    xs = np.ascontiguousarray(
        x.reshape(B, C, 2, H // 2, W).swapaxes(1, 2).reshape(NDEV, C, H // 2, W))

    t0 = time.perf_counter()
    out_sh = _pfwd(xs, wq, bq, wk, bk, wv, bv, wp, bp, lepe_w, lepe_b)
    out_sh = jax.block_until_ready(out_sh)
    LAST_EXEC_NS = (time.perf_counter() - t0) * 1e9

    # slabs [8, C, H/2, W] -> [B, C, H, W]
    out = np.asarray(out_sh)
    out = (out.reshape(B, 2, C, H // 2, W).swapaxes(1, 2)
              .reshape(B, C, H, W))
    return np.ascontiguousarray(out)
